# revision 1
# baseline (speedup 1.0000x reference)
"""EnsembleMLP fused kernel for Trainium2, 8 NeuronCores (SPMD, batch-parallel).

Math transformation
-------------------
reference:
    hidden = tanh(x @ W_in.T)                                   [B, H]
    feats[b,m,e] = hidden[b, ids[m,e]]                          [B, M, E]
    preds[b,m,o] = sum_e feats[b,m,e] * W_pred[m,o,e]           [B, M, O]
    out = preds.mean(axis=1)                                    [B, O]

The gather + per-member matmul + mean are all linear in `hidden`, so they
collapse into a single [H, O] matrix:
    A[h,o] = (1/M) * sum_{(m,e): ids[m,e]==h} W_pred[m,o,e]
    out    = tanh(x @ W_in.T) @ A

A is built on the host from the tiny W_pred/ids tensors (0.7 MB); the device
does the two matmuls + tanh. Sharding: data-parallel over batch — each of the
8 cores takes 512 rows of x; W_in^T and A are replicated. No collectives.

Device layout (per core)
------------------------
All DRAM inputs are host-packed partition-major ([128, free]) so every DMA
moves >=1KB-contiguous per-partition segments:
  xt  [128, 4*512]   bf16  x^T slice:  xt[p, n*512+b] = x[c*512+b, n*128+p]
  wt  [128, 32*512]  bf16  W_in^T:     wt[p, t*512+n*128+h] = W_in[t*128+h, n*128+p]
  aw  [128, 32*10]   bf16  A packed:   aw[p, t*10+o] = A[t*128+p, o]
  out [10, 512]      f32   out^T slice (host transposes back)

  H^T tile [h=128, b=512] = (wt chunk).T @ (xt chunk), accum over 4 i-chunks
  tanh on ACT engine PSUM->SBUF (bf16)
  out^T [10, 512] = sum over 32 h-tiles of (A chunk).T @ H^T tile, 2-way
  column-tiled on the PE (M=10 uses only 10/128 PE columns), final add on DVE.
"""

import os

import numpy as np
import ml_dtypes

BATCH, IN_DIM, HIDDEN, N_MEMBERS, ENS, OUT = 4096, 512, 4096, 256, 64, 10
NCORES = 8
B_LOC = BATCH // NCORES      # 512 batch rows per core
HT = 128                     # h-tile height (PSUM partition dim)
NHT = HIDDEN // HT           # 32 h-tiles
NIC = IN_DIM // 128          # 4 contraction chunks for the first matmul
N_WARM = 38                  # 128-col warm-up matmuls to lift the PE HAM clock-gate
# wt DMA group sizes (in h-tiles). Small leading groups let the first matmuls
# start early; larger trailing groups amortize per-DMA issue cost.
WT_GROUPS = [1] * 8 + [2] * 4 + [4] * 4
assert sum(WT_GROUPS) == NHT

_compiled = None
LAST_RESULT = None           # BassKernelResults of the most recent run


def _build_raw():
    """Hand-scheduled Bass version (no Tile framework).

    Tile's fixed prologue (sem-init barriers) and epilogue (per-sem drain +
    reset butterfly) cost ~12us on a ~50us kernel. This version uses 6
    manual semaphores, one DMA ring (Sync/HWDGE, FIFO — so a single
    cumulative completion counter is sound), and a single all-engine
    barrier + sem clears at the end.

    Engine programs:
      Sync   : xt DMA, 32 wt-chunk DMAs, aw DMA; final out DMA
      GpSimd : memset of the zero tile used for PE warm-up
      PE     : N_WARM warm-up matmuls (HAM clock-gate), 32x4 first-layer
               matmuls (psum bank t%4), then 32 ensemble matmuls 2-way
               column-tiled into 2 PSUM banks
      ACT    : tanh PSUM->SBUF (bf16) per h-tile; dummy op up-front to pull
               the ~1.3us activation-table load off the critical path
      DVE    : copy + add of the two ensemble PSUM accumulators, f32
    """
    from concourse import bacc, mybir

    bf16 = mybir.dt.bfloat16
    f32 = mybir.dt.float32

    nc = bacc.Bacc(
        "TRN2",
        target_bir_lowering=False,
        debug=False,
        enable_asserts=False,
        num_devices=NCORES,
    )
    xt = nc.dram_tensor("xt", [128, NIC * B_LOC], bf16, kind="ExternalInput")
    wt = nc.dram_tensor("wt", [128, NHT * NIC * HT], bf16, kind="ExternalInput")
    aw = nc.dram_tensor("aw", [128, NHT * OUT], bf16, kind="ExternalInput")
    out = nc.dram_tensor("out", [OUT, B_LOC], f32, kind="ExternalOutput")

    warm_sb = nc.alloc_sbuf_tensor("warm_sb", [128, 128], mybir.dt.uint16)
    dummy_sb = nc.alloc_sbuf_tensor("dummy_sb", [1, 16], f32)
    xt_sb = nc.alloc_sbuf_tensor("xt_sb", [128, NIC, B_LOC], bf16)
    wt_sb = [
        nc.alloc_sbuf_tensor(f"wt_g{g}", [128, k, NIC, HT], bf16)
        for g, k in enumerate(WT_GROUPS)
    ]
    ht_sb = [
        nc.alloc_sbuf_tensor(f"ht_sb{t}", [128, B_LOC], bf16) for t in range(NHT)
    ]
    a_sb = nc.alloc_sbuf_tensor("a_sb", [128, NHT * OUT], bf16)
    out_sb = nc.alloc_sbuf_tensor("out_sb", [OUT, B_LOC], f32)

    ps = [nc.alloc_psum_tensor(f"ps{k}", [128, B_LOC], f32) for k in range(4)]
    pso = [nc.alloc_psum_tensor(f"pso{j}", [128, B_LOC], f32) for j in range(2)]
    psw = nc.alloc_psum_tensor("psw", [128, B_LOC], f32)

    # Per-DMA completion semaphores: a DMA's +16 lands only on its own sem,
    # so waits are sound under any cross-DMA completion interleaving (and
    # CoreSim's race detector agrees).
    s_xt = [nc.alloc_semaphore(f"s_xt{i}") for i in range(NIC)]
    s_wt = [nc.alloc_semaphore(f"s_wtg{g}") for g in range(len(WT_GROUPS))]
    s_aw = nc.alloc_semaphore("s_aw")
    s_out = nc.alloc_semaphore("s_out")
    sg = nc.alloc_semaphore("sg")    # zero tile memset done
    sm = nc.alloc_semaphore("sm")    # first-layer tile t accumulated
    sa = nc.alloc_semaphore("sa")    # tanh t done
    sm2 = nc.alloc_semaphore("sm2")  # ensemble matmul count
    sv = nc.alloc_semaphore("sv")    # final add done
    sems = [*s_xt, *s_wt, s_aw, s_out, sg, sm, sa, sm2, sv]

    # tile index -> (group, index within group, group's completion sem)
    tile_group = []
    for g, k in enumerate(WT_GROUPS):
        for i in range(k):
            tile_group.append((g, i))

    tanh = mybir.ActivationFunctionType.Tanh

    # ---- Warm-up tile. The PE HAM clock-gate watches real datapath
    # activity — matmuls on zeros do NOT count as busy — so the warm-up
    # needs varying nonzero data: random bits masked to bf16 in [1, 2)
    # ((bits & 0x7F) | 0x3F80). DVE's preamble finishes ~2us before
    # GpSimd's, so generate it there.
    if os.environ.get("KERNEL_SIMSAFE") == "1":
        fill = nc.vector.memset(warm_sb.ap(), 0x3F80)  # CoreSim xorwow workaround
    else:
        fill = nc.vector.random(warm_sb.ap())
    fill.then_inc(sg, 1)
    nc.vector.wait_ge(sg, 1)              # DVE pipeline: fill retired
    nc.vector.tensor_scalar(
        out=warm_sb.ap(),
        in0=warm_sb.ap(),
        scalar1=0x007F,
        scalar2=0x3F80,
        op0=mybir.AluOpType.bitwise_and,
        op1=mybir.AluOpType.bitwise_or,
    ).then_inc(sg, 1)

    # ---- Input DMAs on BOTH HWDGE rings (Sync + Scalar) in consumption
    # order: each ring is FIFO, so two rings double the issue rate and let
    # the front of the stream land sooner. Even wt groups + xt half 0 on
    # Sync; odd groups + xt half 1 + aw on Scalar.
    xt_view = xt.ap().rearrange("p (n b) -> p n b", n=NIC)
    wt_view = wt.ap().rearrange("p (t n h) -> p t n h", t=NHT, n=NIC)
    group_t0 = []
    t0 = 0
    for k in WT_GROUPS:
        group_t0.append(t0)
        t0 += k
    def wt_dma(eng, g):
        k = WT_GROUPS[g]
        eng.dma_start(
            out=wt_sb[g].ap(), in_=wt_view[:, group_t0[g] : group_t0[g] + k, :, :]
        ).then_inc(s_wt[g], 16)

    def xt_dma(eng, i):
        eng.dma_start(
            out=xt_sb.ap()[:, i, :], in_=xt_view[:, i, :]
        ).then_inc(s_xt[i], 16)

    # Ring A (Sync): wt group 0 first so the very first matmul can start,
    # then xt quarters 0/2 and the even wt groups.
    wt_dma(nc.sync, 0)
    xt_dma(nc.sync, 0)
    wt_dma(nc.sync, 2)
    xt_dma(nc.sync, 2)
    for g in range(4, len(WT_GROUPS), 2):
        wt_dma(nc.sync, g)
    # Ring B (Scalar): xt quarters 1/3 interleaved with the odd wt groups.
    xt_dma(nc.scalar, 1)
    wt_dma(nc.scalar, 1)
    xt_dma(nc.scalar, 3)
    for g in range(3, len(WT_GROUPS), 2):
        wt_dma(nc.scalar, g)
    nc.scalar.dma_start(out=a_sb.ap(), in_=aw.ap()).then_inc(s_aw, 16)

    # ---- PE
    pe = nc.tensor
    pe.wait_ge(sg, 2)
    warm_bf = warm_sb.ap().bitcast(bf16)
    for _ in range(N_WARM):
        pe.matmul(
            out=psw.ap()[:, :128],
            lhsT=warm_bf,
            rhs=warm_bf,
            start=True,
            stop=True,
        )
    for t in range(NHT):
        g, i = tile_group[t]
        if i == 0:
            pe.wait_ge(s_wt[g], 16)           # wt group g landed
        if t >= 4:
            pe.wait_ge(sa, t - 3)             # psum bank free after tanh(t-4)
        for n in range(NIC):
            if t == 0:
                pe.wait_ge(s_xt[n], 16)       # xt quarter n landed
            mm = pe.matmul(
                out=ps[t % 4].ap(),
                lhsT=wt_sb[g].ap()[:, i, n, :],
                rhs=xt_sb.ap()[:, n, :],
                start=(n == 0),
                stop=(n == NIC - 1),
            )
        mm.then_inc(sm, 1)
    pe.wait_ge(s_aw, 16)                      # aw landed
    for t in range(NHT):
        j = t % 2
        pe.wait_ge(sa, t + 1)                 # ht tile t written
        pe.matmul(
            out=pso[j].ap()[64 * j : 64 * j + OUT, :],
            lhsT=a_sb.ap()[:, t * OUT : (t + 1) * OUT],
            rhs=ht_sb[t].ap(),
            start=(t < 2),
            stop=(t >= NHT - 2),
            tile_position=(0, 64 * j),
        ).then_inc(sm2, 1)

    # ---- ACT: dummy first use pulls the act-table load off the critical path
    act = nc.scalar
    act.wait_ge(sg, 2)
    act.activation(
        out=dummy_sb.ap(), in_=warm_sb.ap().bitcast(bf16)[:1, :16], func=tanh
    )
    for t in range(NHT):
        act.wait_ge(sm, t + 1)
        act.activation(out=ht_sb[t].ap(), in_=ps[t % 4].ap(), func=tanh).then_inc(
            sa, 1
        )
    act.wait_ge(sm2, NHT - 1)                 # column group 0 finished (t=30)
    act.activation(
        out=out_sb.ap(),
        in_=pso[0].ap()[0:OUT, :],
        func=mybir.ActivationFunctionType.Copy,
    ).then_inc(sv, 1)

    # ---- DVE: add the second ensemble accumulator
    v = nc.vector
    v.wait_ge(sm2, NHT)
    v.wait_ge(sv, 1)                          # ACT copy done
    v.tensor_add(
        out=out_sb.ap(),
        in0=out_sb.ap(),
        in1=pso[1].ap()[64 : 64 + OUT, :],
    ).then_inc(sv, 1)

    # ---- Sync tail: result out. No explicit completion wait or sem reset:
    # the NRT-injected per-engine epilogue drains every queue and resets the
    # whole semaphore file after the program ends, so both would only add
    # ~3us of counted exec time.
    nc.sync.wait_ge(sv, 2)
    nc.sync.dma_start(out=out.ap(), in_=out_sb.ap()).then_inc(s_out, 16)

    nc.compile()
    return nc


def _build():
    from concourse import bacc, mybir
    import concourse.tile as tile

    bf16 = mybir.dt.bfloat16
    f32 = mybir.dt.float32

    nc = bacc.Bacc(
        "TRN2",
        target_bir_lowering=False,
        debug=False,
        enable_asserts=False,
        num_devices=NCORES,
    )
    xt = nc.dram_tensor("xt", [128, NIC * B_LOC], bf16, kind="ExternalInput")
    wt = nc.dram_tensor("wt", [128, NHT * NIC * HT], bf16, kind="ExternalInput")
    aw = nc.dram_tensor("aw", [128, NHT * OUT], bf16, kind="ExternalInput")
    out = nc.dram_tensor("out", [OUT, B_LOC], f32, kind="ExternalOutput")

    with tile.TileContext(nc) as tc:
        with (
            tc.tile_pool(name="single", bufs=1) as single,
            tc.tile_pool(name="wpool", bufs=NHT) as wpool,
            tc.tile_pool(name="hpool", bufs=NHT) as hpool,
            tc.tile_pool(name="ps", bufs=4, space="PSUM") as pspool,
            tc.tile_pool(name="psout", bufs=1, space="PSUM") as psoutp,
            tc.tile_pool(name="pswarm", bufs=1, space="PSUM") as pswarm,
        ):
            # PE warm-up: the HAM clock gate holds the PE at 1.2 GHz until it
            # has been busy ~3.4us. Burn that window on zeros while the input
            # DMAs are still in flight so the real matmuls all run at 2.4 GHz.
            zero_sb = single.tile([128, B_LOC], bf16)
            nc.vector.memset(zero_sb[:], 0.0)
            ps_w = pswarm.tile([128, B_LOC], f32)
            for _ in range(N_WARM):
                nc.tensor.matmul(
                    out=ps_w[:],
                    lhsT=zero_sb[:, :128],
                    rhs=zero_sb[:],
                    start=True,
                    stop=True,
                )

            # x^T chunks: 4 separate tiles/DMAs so matmuls can start after the
            # first lands. (Multiple DMAs into slices of ONE tile hang the HW.)
            xt_view = xt.ap().rearrange("p (n b) -> p n b", n=NIC)
            xt_tiles = []
            for n in range(NIC):
                xt_n = single.tile([128, B_LOC], bf16, name=f"xt{n}")
                nc.sync.dma_start(out=xt_n[:], in_=xt_view[:, n, :])
                xt_tiles.append(xt_n)

            # wt in per-h-tile chunks (128 KB) so the first matmul can start
            # as soon as chunk 0 lands instead of after the whole 4 MB load.
            # Each HWDGE dma_start occupies the Sync queue ~600ns, so issue
            # odd chunks from GpSimd (SWDGE) to halve the issue serialization.
            wt_view = wt.ap().rearrange("p (t n h) -> p t n h", t=NHT, n=NIC)
            wt_tiles = []
            for t in range(NHT):
                wt_t = wpool.tile([128, NIC, HT], bf16)
                eng = nc.sync if t % 2 == 0 else nc.gpsimd
                eng.dma_start(out=wt_t[:], in_=wt_view[:, t, :, :])
                wt_tiles.append(wt_t)

            # aw is only needed by the trailing ensemble matmul: issue last.
            a_sb = single.tile([128, NHT * OUT], bf16)
            nc.gpsimd.dma_start(out=a_sb[:], in_=aw.ap())

            # hidden^T tiles: H^T[t*128+p, b] = tanh(sum_i W[h,i] x[b,i])
            ht_tiles = []
            for t in range(NHT):
                ps = pspool.tile([128, B_LOC], f32)
                for n in range(NIC):
                    nc.tensor.matmul(
                        out=ps[:],
                        lhsT=wt_tiles[t][:, n, :],
                        rhs=xt_tiles[n][:],
                        start=(n == 0),
                        stop=(n == NIC - 1),
                    )
                ht = hpool.tile([128, B_LOC], bf16)
                nc.scalar.activation(
                    out=ht[:], in_=ps[:], func=mybir.ActivationFunctionType.Tanh
                )
                ht_tiles.append(ht)

            # out^T[o, b] = sum_t (A chunk t).T @ H^T tile t. M=10 uses only
            # 10/128 PE columns, so run 2 h-tiles concurrently in distinct
            # column groups (tile_position 0 / 64), each accumulating into
            # its own PSUM bank at the matching partition offset.
            ps_outs = [
                psoutp.tile([128, B_LOC], f32, name=f"ps_out{j}") for j in range(2)
            ]
            for t in range(NHT):
                j = t % 2
                nc.tensor.matmul(
                    out=ps_outs[j][64 * j : 64 * j + OUT, :],
                    lhsT=a_sb[:, t * OUT : (t + 1) * OUT],
                    rhs=ht_tiles[t][:],
                    start=(t < 2),
                    stop=(t >= NHT - 2),
                    tile_position=(0, 64 * j),
                )
            out_sb = single.tile([OUT, B_LOC], f32)
            nc.vector.tensor_copy(out=out_sb[:], in_=ps_outs[0][0:OUT, :])
            nc.vector.tensor_add(
                out=out_sb[:],
                in0=out_sb[:],
                in1=ps_outs[1][64 : 64 + OUT, :],
            )
            nc.sync.dma_start(out=out.ap(), in_=out_sb[:])

    nc.compile()
    return nc


def kernel(**inputs) -> np.ndarray:
    x = np.asarray(inputs["x"], dtype=np.float32)              # [4096, 512]
    W_in = np.asarray(inputs["W_in"], dtype=np.float32)        # [4096, 512]
    W_pred = np.asarray(inputs["W_pred"], dtype=np.float32)    # [256, 10, 64]
    ids = np.asarray(inputs["ensemble_input_ids"])             # [256, 64] int32

    # Collapse gather + einsum + mean into A[h, o].
    A = np.zeros((HIDDEN, OUT), dtype=np.float64)
    np.add.at(
        A,
        ids.reshape(-1),
        W_pred.transpose(0, 2, 1).reshape(-1, OUT).astype(np.float64),
    )
    A /= N_MEMBERS
    a_packed = np.ascontiguousarray(
        A.reshape(NHT, 128, OUT).transpose(1, 0, 2).reshape(128, NHT * OUT)
    ).astype(ml_dtypes.bfloat16)

    xt_bf = x.T.astype(ml_dtypes.bfloat16)                     # [512, 4096]
    wt_bf = W_in.T.astype(ml_dtypes.bfloat16)                  # [512, 4096]
    # wt packed partition-major: [p, t*512 + n*128 + h] = W_in.T[n*128+p, t*128+h]
    wt_packed = np.ascontiguousarray(
        wt_bf.reshape(NIC, 128, NHT, HT).transpose(1, 2, 0, 3).reshape(128, -1)
    )

    global _compiled
    if _compiled is None:
        if os.environ.get("KERNEL_IMPL", "raw") == "tile":
            _compiled = _build()
        else:
            _compiled = _build_raw()
    nc = _compiled

    in_maps = []
    for c in range(NCORES):
        xs = xt_bf[:, c * B_LOC : (c + 1) * B_LOC]             # [512, 512]
        xt_packed = np.ascontiguousarray(
            xs.reshape(NIC, 128, B_LOC).transpose(1, 0, 2).reshape(128, -1)
        )
        in_maps.append({"xt": xt_packed, "wt": wt_packed, "aw": a_packed})

    from concourse.bass_utils import run_bass_kernel_spmd

    trace = bool(int(os.environ.get("KERNEL_TRACE", "0")))
    res = run_bass_kernel_spmd(
        nc, in_maps, core_ids=list(range(NCORES)), trace=trace
    )
    global LAST_RESULT
    LAST_RESULT = res

    out = np.empty((BATCH, OUT), dtype=np.float32)
    for c in range(NCORES):
        out[c * B_LOC : (c + 1) * B_LOC, :] = res.results[c]["out"].T
    return out

